# revision 1
# baseline (speedup 1.0000x reference)
import sys
import numpy as np

sys.path.insert(0, "/opt/trn_rl_repo")

import concourse.bass as bass  # noqa: E402
import concourse.tile as tile  # noqa: E402
from concourse import bacc, mybir  # noqa: E402
from concourse.ap import AP  # noqa: E402
from concourse.bass_utils import run_bass_kernel_spmd  # noqa: E402
import ml_dtypes  # noqa: E402

BF16 = mybir.dt.bfloat16
F32 = mybir.dt.float32
FP8 = mybir.dt.float8e4
DRMODE = mybir.MatmulPerfMode.DoubleRow
DIM = 70
HW = DIM * DIM  # 4900
CUBE = DIM * HW  # 343000

_CACHE = {}


def _build():
    nc = bacc.Bacc("TRN2", target_bir_lowering=False, debug=False, num_devices=8)
    xin_d = nc.dram_tensor("xin", [70, 6 * HW], BF16, kind="ExternalInput")
    gt_d = nc.dram_tensor("gt", [70, 6 * 70], BF16, kind="ExternalInput")
    w1a_d = nc.dram_tensor("w1a", [118, 32], BF16, kind="ExternalInput")
    w1c_d = nc.dram_tensor("w1c", [118, 32], BF16, kind="ExternalInput")
    w1b_d = nc.dram_tensor("w1b", [54, 32], BF16, kind="ExternalInput")
    w1e_d = nc.dram_tensor("w1e", [54, 32], BF16, kind="ExternalInput")
    w2_d = nc.dram_tensor("w2", [96, 576], BF16, kind="ExternalInput")
    w3a_d = nc.dram_tensor("w3a", [128, 9 * 128], BF16, kind="ExternalInput")
    w3b_d = nc.dram_tensor("w3b", [64, 9 * 128], BF16, kind="ExternalInput")
    w4_d = nc.dram_tensor("w4", [128, 27 * 256], BF16, kind="ExternalInput")
    f1_d = nc.dram_tensor("f1", [128, 16 * 1024], BF16, kind="ExternalInput")
    f2_d = nc.dram_tensor("f2", [128, 8 * 29], BF16, kind="ExternalInput")
    b1_d = nc.dram_tensor("b1", [128, 1], F32, kind="ExternalInput")
    b2_d = nc.dram_tensor("b2", [128, 1], F32, kind="ExternalInput")
    b3_d = nc.dram_tensor("b3", [128, 1], F32, kind="ExternalInput")
    b4_d = nc.dram_tensor("b4", [128, 2], F32, kind="ExternalInput")
    fb1_d = nc.dram_tensor("fb1", [128, 8], F32, kind="ExternalInput")
    fb2_d = nc.dram_tensor("fb2", [29, 1], F32, kind="ExternalInput")
    y_d = nc.dram_tensor("y", [29], F32, kind="ExternalOutput")
    cube_d = nc.dram_tensor("cube", [6 * CUBE + 512], BF16)

    Relu = mybir.ActivationFunctionType.Relu
    Copy = mybir.ActivationFunctionType.Copy
    amax = mybir.AluOpType.max
    aadd = mybir.AluOpType.add

    with tile.TileContext(nc, pool_alloc_mode="queue") as tc:
        with (
            tc.tile_pool(name="const", bufs=1) as constp,
        ):
            gt = constp.tile([70, 6 * 70], BF16)
            nc.sync.dma_start(gt[:], gt_d[:])
            w1a = constp.tile([118, 32], BF16)
            nc.sync.dma_start(w1a[:], w1a_d[:])
            w1c = constp.tile([118, 32], BF16)
            nc.sync.dma_start(w1c[:], w1c_d[:])
            w1b = constp.tile([54, 32], BF16)
            nc.sync.dma_start(w1b[:], w1b_d[:])
            w1e = constp.tile([118, 32], BF16)
            nc.sync.dma_start(w1e[64:118, :], w1e_d[:])
            w2 = constp.tile([96, 576], BF16)
            nc.sync.dma_start(w2[:], w2_d[:])
            b1 = constp.tile([128, 1], F32)
            nc.sync.dma_start(b1[:], b1_d[:])
            b2 = constp.tile([128, 1], F32)
            nc.sync.dma_start(b2[:], b2_d[:])
            b3 = constp.tile([128, 1], F32)
            nc.sync.dma_start(b3[:], b3_d[:])
            b4 = constp.tile([128, 2], F32)
            nc.sync.dma_start(b4[:], b4_d[:])
            fb1 = constp.tile([128, 8], F32)
            nc.sync.dma_start(fb1[:], fb1_d[:])
            fb2 = constp.tile([29, 1], F32)
            nc.sync.dma_start(fb2[:], fb2_d[:])

            # ---------------- blur ----------------
            with (
                tc.tile_pool(name="xinp", bufs=1) as xinp,
                tc.tile_pool(name="t12", bufs=4) as t12p,
                tc.tile_pool(name="cst", bufs=3) as cstp,
                tc.tile_pool(name="bps", bufs=4, space="PSUM") as bps,
            ):
                xin = xinp.tile([70, 6 * HW], BF16)
                nc.sync.dma_start(xin[:], xin_d[:])
                xr = xin[:].rearrange("p (e j k) -> p e j k", e=6, j=70, k=70)
                for e in range(6):
                    ge = gt[:, e * 70:(e + 1) * 70]
                    t1 = t12p.tile([70, HW], BF16, tag="t")
                    # stage A: contract i -> T1[j, (k,a)]
                    for g in range(10):
                        ps = bps.tile([70, 490], F32, tag="ps")
                        for s in range(7):
                            k = g * 7 + s
                            nc.tensor.matmul(ps[:, s * 70:(s + 1) * 70],
                                             xr[:, e, :, k], ge)
                        if g % 2 == 0:
                            nc.scalar.activation(t1[:, g * 490:(g + 1) * 490], ps[:], Copy)
                        else:
                            nc.vector.tensor_copy(t1[:, g * 490:(g + 1) * 490], ps[:])
                    t1r = t1[:].rearrange("p (k a) -> p k a", k=70)
                    t2 = t12p.tile([70, HW], BF16, tag="t")
                    # stage B: contract j -> T2[k, (a,p)]
                    for g in range(10):
                        ps = bps.tile([70, 490], F32, tag="ps")
                        for s in range(7):
                            a = g * 7 + s
                            nc.tensor.matmul(ps[:, s * 70:(s + 1) * 70],
                                             t1r[:, :, a], ge)
                        if g % 2 == 0:
                            nc.scalar.activation(t2[:, g * 490:(g + 1) * 490], ps[:], Copy)
                        else:
                            nc.vector.tensor_copy(t2[:, g * 490:(g + 1) * 490], ps[:])
                    # stage C: contract k -> cube[(a,p) chunks, q]
                    cst = cstp.tile([128, 39 * 70], BF16)
                    for g in range(6):
                        nch = 7 if g < 5 else 4
                        ps = bps.tile([128, 490], F32, tag="psc")
                        for s in range(nch):
                            c = g * 7 + s
                            cw = 128 if c < 38 else 36
                            nc.tensor.matmul(ps[:cw, s * 70:(s + 1) * 70],
                                             t2[:, c * 128:c * 128 + cw], ge)
                        w_ = nch * 70
                        if g % 2 == 0:
                            nc.scalar.activation(cst[:, g * 490:g * 490 + w_], ps[:, :w_], Copy)
                        else:
                            nc.vector.tensor_copy(cst[:, g * 490:g * 490 + w_], ps[:, :w_])
                    # DMA cst -> cube[e]: chunks 0..37 full (128 rows), chunk 38 partial (36)
                    cr = cst[:].rearrange("p (c q) -> p c q", c=39)
                    d1 = AP(cube_d, e * CUBE, [[70, 128], [8960, 38], [1, 70]])
                    nc.sync.dma_start(d1, cr[:, 0:38, :])
                    d2 = AP(cube_d, e * CUBE + 38 * 8960, [[70, 36], [1, 70]])
                    nc.sync.dma_start(d2, cr[0:36, 38, :])

            h3p = tc.alloc_tile_pool(name="h3p", bufs=1)
            # rows 0:64 = H3[z]; rows 64:128 = H3[z+1] (dz-pair for conv3 K=128)
            H3 = h3p.tile([128, 16 * 16 * 16], BF16)
            h4p = tc.alloc_tile_pool(name="h4p", bufs=1)
            H4 = h4p.tile([128, 343], BF16)
            # ---------------- conv1 (+pool+relu) ----------------
            h2p = tc.alloc_tile_pool(name="h2p", bufs=1)
            H2 = h2p.tile([96, 34 * 34 * 34], BF16)
            L34 = 34 * 34 * 34
            with (
                tc.tile_pool(name="ring", bufs=8) as ringp,
                tc.tile_pool(name="c1ps", bufs=8, space="PSUM") as c1ps,
                tc.tile_pool(name="c1tmp", bufs=3) as c1tmp,
            ):
                slices = {}

                def load_slice(s):
                    # even-pair tile: rows 0:54 = slice s, rows 64:118 = slice s+1,
                    # rows 54:64 = finite pad (matched by zero weight rows)
                    t = ringp.tile([118, HW], BF16, tag="sl")
                    for half, base in ((0, 0), (1, 64)):
                        sp = s + half
                        for dy in range(3):
                            src = AP(cube_d, sp * HW + 70 * dy, [[1, 3], [CUBE, 6], [1, HW]])
                            nc.sync.dma_start(t[base + dy * 18:base + (dy + 1) * 18, :], src)
                    pad = AP(cube_d, s * HW, [[1, 2], [CUBE, 5], [1, HW]])
                    nc.sync.dma_start(t[54:64, :], pad)
                    slices[s] = t

                for s in (0, 2, 4):
                    load_slice(s)
                for w in range(17):
                    z0 = 4 * w
                    for s in (z0 + 6, z0 + 8):
                        if s <= 68:
                            load_slice(s)
                    wt = c1tmp.tile([128, 3672], BF16, tag="tmp")
                    for c in range(12):
                        ny = 6 if c < 11 else 2
                        N = ny * 68
                        ps = c1ps.tile([128, 408], F32, tag="ps")
                        for g in range(4):
                            z = z0 + g
                            if z % 2 == 0:
                                tA = slices[z]
                                tC = slices[z + 2]
                                rhs1 = tA[:].rearrange("p (y x) -> p y x", y=70)[:, 6 * c:6 * c + ny, 0:68]
                                nc.tensor.matmul(ps[g * 32:(g + 1) * 32, :N],
                                                 w1a[:], rhs1, start=True, stop=False,
                                                 tile_position=(0, 32 * g))
                                rhs2 = tC[0:54, :].rearrange("p (y x) -> p y x", y=70)[:, 6 * c:6 * c + ny, 0:68]
                                nc.tensor.matmul(ps[g * 32:(g + 1) * 32, :N],
                                                 w1b[:], rhs2, start=False, stop=True,
                                                 tile_position=(0, 32 * g))
                            else:
                                tA = slices[z - 1]
                                tB = slices[z + 1]
                                rhs1 = tA[64:118, :].rearrange("p (y x) -> p y x", y=70)[:, 6 * c:6 * c + ny, 0:68]
                                nc.tensor.matmul(ps[g * 32:(g + 1) * 32, :N],
                                                 w1e[64:118, :], rhs1, start=True, stop=False,
                                                 tile_position=(64, 32 * g))
                                rhs2 = tB[:].rearrange("p (y x) -> p y x", y=70)[:, 6 * c:6 * c + ny, 0:68]
                                nc.tensor.matmul(ps[g * 32:(g + 1) * 32, :N],
                                                 w1c[:], rhs2, start=False, stop=True,
                                                 tile_position=(0, 32 * g))
                        pr = ps[:].rearrange("p (y xp two) -> p y xp two", y=6, two=2)
                        nyv = ny
                        nc.vector.tensor_reduce(
                            wt[:, c * 204:c * 204 + nyv * 34].rearrange("p (y x) -> p y x", y=nyv),
                            pr[:, 0:nyv, :, :], mybir.AxisListType.X, amax)
                    # y-pool over pairs within chunks (into tail of wt)
                    yp = wt[:, 2448:3672]
                    wr = wt[:, 0:2448].rearrange("p (c yp two x) -> p c yp two x", c=12, yp=3, two=2)
                    ypr = yp.rearrange("p (c yp x) -> p c yp x", c=12, yp=3)
                    nc.vector.tensor_tensor(ypr[:], wr[:, :, :, 0, :], wr[:, :, :, 1, :], amax)
                    # regroup partition groups into free dim, then z-pool + bias + relu
                    ypb = c1tmp.tile([32, 4 * 1224], BF16, tag="tmp")
                    for g in range(4):
                        nc.sync.dma_start(ypb[:, g * 1224:(g + 1) * 1224],
                                          yp[g * 32:(g + 1) * 32, :])
                    for zp in range(2):
                        zq = 2 * w + zp
                        hsl = H2[0:32, zq * 1156:(zq + 1) * 1156]
                        nc.vector.tensor_tensor(hsl, ypb[:, zp * 2448:zp * 2448 + 1156],
                                                ypb[:, zp * 2448 + 1224:zp * 2448 + 2380], amax)
                        nc.scalar.activation(hsl, hsl, Relu, bias=b1[0:32, :])
                        if zq >= 1:
                            nc.sync.dma_start(H2[32:64, (zq - 1) * 1156:zq * 1156], hsl)
                        if zq >= 2:
                            nc.sync.dma_start(H2[64:96, (zq - 2) * 1156:(zq - 1) * 1156], hsl)

            # ---------------- conv2 ----------------
            with (
                tc.tile_pool(name="c2ps", bufs=6, space="PSUM") as c2ps,
                tc.tile_pool(name="c2tmp", bufs=3) as c2tmp,
            ):
                h2r = H2[:].rearrange("p (z y x) -> p z y x", z=34, y=34)
                for w in range(16):
                    z0 = 2 * w
                    wt = c2tmp.tile([128, 512], BF16, tag="wt")
                    for c in range(2):
                        ps = c2ps.tile([128, 512], F32, tag="ps")
                        for t in range(9):
                            dy, dx = t // 3, t % 3
                            for g in range(2):
                                rhs = h2r[:, z0 + g, c * 16 + dy:c * 16 + dy + 16, dx:dx + 32]
                                nc.tensor.matmul(ps[g * 64:(g + 1) * 64, :],
                                                 w2[:, t * 64:(t + 1) * 64], rhs,
                                                 start=(t == 0), stop=(t == 8),
                                                 tile_position=(0, 64 * g))
                        pr = ps[:].rearrange("p (y xp two) -> p y xp two", y=16, two=2)
                        nc.vector.tensor_reduce(
                            wt[:, c * 256:(c + 1) * 256].rearrange("p (y x) -> p y x", y=16),
                            pr[:], mybir.AxisListType.X, amax)
                    yp = c2tmp.tile([128, 256], BF16, tag="yp")
                    wr = wt[:].rearrange("p (c yp two x) -> p c yp two x", c=2, yp=8, two=2)
                    ypr = yp[:].rearrange("p (c yp x) -> p c yp x", c=2, yp=8)
                    nc.vector.tensor_tensor(ypr[:], wr[:, :, :, 0, :], wr[:, :, :, 1, :], amax)
                    ypb = c2tmp.tile([64, 512], BF16, tag="ypb")
                    for g in range(2):
                        nc.sync.dma_start(ypb[:, g * 256:(g + 1) * 256],
                                          yp[g * 64:(g + 1) * 64, :])
                    hsl = H3[0:64, w * 256:(w + 1) * 256]
                    nc.vector.tensor_tensor(hsl, ypb[:, 0:256], ypb[:, 256:512], amax)
                    nc.scalar.activation(hsl, hsl, Relu, bias=b2[0:64, :])
                    if w >= 1:
                        nc.sync.dma_start(H3[64:128, (w - 1) * 256:w * 256], hsl)

            h2p.release()
            # ---------------- conv3 ----------------
            fcp = tc.alloc_tile_pool(name="fcp", bufs=1)
            w3a = fcp.tile([128, 9 * 128], BF16)
            nc.sync.dma_start(w3a[:], w3a_d[:])
            w3b = fcp.tile([64, 9 * 128], BF16)
            nc.sync.dma_start(w3b[:], w3b_d[:])
            w4 = fcp.tile([128, 27 * 256], BF16)
            nc.sync.dma_start(w4[:], w4_d[:])
            f1 = fcp.tile([128, 16 * 1024], BF16)
            nc.sync.dma_start(f1[:], f1_d[:])
            f2 = fcp.tile([128, 8 * 29], BF16)
            nc.sync.dma_start(f2[:], f2_d[:])
            with (
                tc.tile_pool(name="c3ps", bufs=8, space="PSUM") as c3ps,
                tc.tile_pool(name="c3tmp", bufs=16) as c3tmp,
            ):
                h3r = H3[:].rearrange("p (z y x) -> p z y x", z=16, y=16)
                zts = {}
                for half in range(2):
                    pss = []
                    for zi7 in range(7):
                        pszz = c3ps.tile([128, 196], F32, tag="ps")
                        pss.append(pszz)
                    for t9 in range(9):
                        dy, dx = t9 // 3, t9 % 3
                        for zi in range(7):
                            z = half * 7 + zi
                            rhs = h3r[:, z, dy:dy + 14, dx:dx + 14]
                            nc.tensor.matmul(pss[zi][:], w3a[:, t9 * 128:(t9 + 1) * 128],
                                             rhs, start=(t9 == 0), stop=False)
                            rhs2 = h3r[0:64, z + 2, dy:dy + 14, dx:dx + 14]
                            nc.tensor.matmul(pss[zi][:], w3b[:, t9 * 128:(t9 + 1) * 128],
                                             rhs2, start=False, stop=(t9 == 8))
                    for zi in range(7):
                        z = half * 7 + zi
                        ps = pss[zi]
                        pr = ps[:].rearrange("p (y xp two) -> p y xp two", y=14, two=2)
                        xt = c3tmp.tile([128, 98], F32, tag="xt")
                        xtr = xt[:].rearrange("p (y x) -> p y x", y=14)
                        nc.vector.tensor_reduce(xtr[:], pr[:], mybir.AxisListType.X, amax)
                        yt = c3tmp.tile([128, 49], F32, tag="yt")
                        ytr = yt[:].rearrange("p (y x) -> p y x", y=7)
                        xr2 = xt[:].rearrange("p (yp two x) -> p yp two x", yp=7, two=2)
                        nc.vector.tensor_tensor(ytr[:], xr2[:, :, 0, :], xr2[:, :, 1, :], amax)
                        zts[z] = yt
                for zq in range(7):
                    zt = c3tmp.tile([128, 49], F32, tag="zt")
                    nc.vector.tensor_tensor(zt[:], zts[2 * zq][:], zts[2 * zq + 1][:], amax)
                    nc.scalar.activation(H4[:, zq * 49:(zq + 1) * 49], zt[:],
                                         Relu, bias=b3[:])

            # ---------------- conv4 + fc ----------------
            with (
                tc.tile_pool(name="c4ps", bufs=2, space="PSUM") as c4ps,
                tc.tile_pool(name="c4tmp", bufs=8) as c4tmp,
            ):
                h4r = H4[:].rearrange("p (z y x) -> p z y x", z=7, y=7)
                v = c4tmp.tile([128, 16], BF16, tag="v")
                for mt in range(2):
                    ps = c4ps.tile([128, 125], F32, tag="ps")
                    for t in range(27):
                        dz, dy, dx = t // 9, (t // 3) % 3, t % 3
                        rhs = h4r[:, dz:dz + 5, dy:dy + 5, dx:dx + 5]
                        nc.tensor.matmul(ps[:], w4[:, t * 256 + mt * 128:t * 256 + (mt + 1) * 128],
                                         rhs, start=(t == 0), stop=(t == 26))
                    pr0 = ps[:].rearrange("p (z y x) -> p z y x", z=5, y=5)
                    pr = pr0[:, :, :, 0:4].rearrange("p z y (xp two) -> p (z y) xp two", two=2)
                    xt = c4tmp.tile([128, 50], F32, tag="xt")
                    xtr = xt[:].rearrange("p (zy x) -> p zy x", x=2)
                    nc.vector.tensor_reduce(xtr[:], pr[:], mybir.AxisListType.X, amax)
                    x20 = xt[:].rearrange("p (z y x) -> p z y x", z=5, y=5)
                    x2 = x20[:, :, 0:4, :].rearrange("p z (yp two) x -> p z yp two x", two=2)
                    yt = c4tmp.tile([128, 20], F32, tag="yt")
                    ytr = yt[:].rearrange("p (z y x) -> p z y x", z=5, y=2)
                    nc.vector.tensor_tensor(ytr[:], x2[:, :, :, 0, :], x2[:, :, :, 1, :], amax)
                    y2r0 = yt[:].rearrange("p (z yx) -> p z yx", z=5)
                    y2r = y2r0[:, 0:4, :].rearrange("p (zp two) yx -> p zp two yx", two=2)
                    zt = c4tmp.tile([128, 8], F32, tag="zt")
                    ztr = zt[:].rearrange("p (z yx) -> p z yx", z=2)
                    nc.vector.tensor_tensor(ztr[:], y2r[:, :, 0, :], y2r[:, :, 1, :], amax)
                    nc.scalar.activation(v[:, mt * 8:(mt + 1) * 8], zt[:],
                                         Relu, bias=b4[:, mt:mt + 1])
                # fc1
                ps5 = c4ps.tile([128, 8], F32, tag="fc1")
                for m in range(8):
                    for kt in range(16):
                        nc.tensor.matmul(ps5[:, m:m + 1],
                                         f1[:, kt * 1024 + m * 128:kt * 1024 + (m + 1) * 128],
                                         v[:, kt:kt + 1],
                                         start=(kt == 0), stop=(kt == 15))
                y1s = c4tmp.tile([128, 8], F32, tag="y1a")
                nc.vector.tensor_tensor(y1s[:], ps5[:], fb1[:], aadd)
                y1b = c4tmp.tile([128, 8], BF16, tag="y1b")
                nc.vector.tensor_scalar_max(y1b[:], y1s[:], 0.0)
                # fc2
                ps6 = c4ps.tile([29, 1], F32, tag="fc2")
                for kt in range(8):
                    nc.tensor.matmul(ps6[:], f2[:, kt * 29:(kt + 1) * 29],
                                     y1b[:, kt:kt + 1],
                                     start=(kt == 0), stop=(kt == 7))
                yout = c4tmp.tile([29, 1], F32, tag="yo")
                nc.vector.tensor_tensor(yout[:], ps6[:], fb2[:], aadd)
                nc.sync.dma_start(AP(y_d, 0, [[1, 29], [1, 1]]), yout[:])
            fcp.release()
            h4p.release()
            h3p.release()
    nc.compile()
    return nc


def _prep(inputs):
    x = np.asarray(inputs["x"], np.float32)
    sigma = np.asarray(inputs["sigma"], np.float32)
    coords = np.arange(DIM, dtype=np.float32) - DIM / 2.0
    idx = np.arange(DIM, dtype=np.float32)
    d2 = (coords[:, None] - idx[None, :]) ** 2
    G = np.exp(-d2[None] / (2.0 * sigma[:, None, None] ** 2))  # [6, a, i]
    gt = np.ascontiguousarray(G.transpose(0, 2, 1)).reshape(6, 70, 70)
    gt_dev = np.zeros((70, 6 * 70), np.float32)
    for e in range(6):
        gt_dev[:, e * 70:(e + 1) * 70] = gt[e]

    w1 = np.asarray(inputs["conv1_w"], np.float32)  # [32,6,3,3,3]
    w1_dev = np.zeros((54, 96), np.float32)
    for dy in range(3):
        for dx in range(3):
            for e in range(6):
                row = (dy * 3 + dx) * 6 + e
                for dz in range(3):
                    w1_dev[row, dz * 32:(dz + 1) * 32] = w1[:, e, dz, dy, dx]
    w1a_dev = np.zeros((118, 32), np.float32)
    w1a_dev[0:54] = w1_dev[:, 0:32]      # dz0
    w1a_dev[64:118] = w1_dev[:, 32:64]   # dz1
    w1c_dev = np.zeros((118, 32), np.float32)
    w1c_dev[0:54] = w1_dev[:, 32:64]     # dz1
    w1c_dev[64:118] = w1_dev[:, 64:96]   # dz2
    w1b_dev = np.ascontiguousarray(w1_dev[:, 64:96])  # dz2
    w1e_dev = np.ascontiguousarray(w1_dev[:, 0:32])   # dz0
    w2 = np.asarray(inputs["conv2_w"], np.float32)  # [64,32,3,3,3]
    w2_dev = np.zeros((96, 576), np.float32)
    for dz in range(3):
        for c in range(32):
            row = dz * 32 + c
            for t in range(9):
                dy, dx = t // 3, t % 3
                w2_dev[row, t * 64:(t + 1) * 64] = w2[:, c, dz, dy, dx]
    w3 = np.asarray(inputs["conv3_w"], np.float32)  # [128,64,3,3,3]
    w3_dev = np.zeros((64, 27 * 128), np.float32)
    for t in range(27):
        dz, dy, dx = t // 9, (t // 3) % 3, t % 3
        w3_dev[:, t * 128:(t + 1) * 128] = w3[:, :, dz, dy, dx].T
    w3a_dev = np.zeros((128, 9 * 128), np.float32)
    w3a_dev[0:64] = w3_dev[:, 0:9 * 128]          # dz0
    w3a_dev[64:128] = w3_dev[:, 9 * 128:18 * 128]  # dz1
    w3b_dev = np.ascontiguousarray(w3_dev[:, 18 * 128:27 * 128])  # dz2
    w4 = np.asarray(inputs["conv4_w"], np.float32)  # [256,128,3,3,3]
    w4_dev = np.zeros((128, 27 * 256), np.float32)
    for t in range(27):
        dz, dy, dx = t // 9, (t // 3) % 3, t % 3
        for mt in range(2):
            w4_dev[:, t * 256 + mt * 128:t * 256 + (mt + 1) * 128] = \
                w4[mt * 128:(mt + 1) * 128, :, dz, dy, dx].T
    fc1w = np.asarray(inputs["fc1_w"], np.float32)  # [1024, 2048]
    f1_dev = np.zeros((128, 16 * 1024), np.float32)
    for kt in range(16):
        mt, vox = kt // 8, kt % 8
        for p in range(128):
            f1_dev[p, kt * 1024:(kt + 1) * 1024] = fc1w[:, (mt * 128 + p) * 8 + vox]
    fc2w = np.asarray(inputs["fc2_w"], np.float32)  # [29, 1024]
    f2_dev = np.zeros((128, 8 * 29), np.float32)
    for kt in range(8):
        f2_dev[:, kt * 29:(kt + 1) * 29] = fc2w[:, kt * 128:(kt + 1) * 128].T

    bf = lambda a: a.astype(ml_dtypes.bfloat16)
    common = dict(
        gt=bf(gt_dev), w1a=bf(w1a_dev), w1b=bf(w1b_dev), w1c=bf(w1c_dev),
        w1e=bf(w1e_dev), w2=bf(w2_dev), w3a=bf(w3a_dev), w3b=bf(w3b_dev), w4=bf(w4_dev),
        f1=bf(f1_dev), f2=bf(f2_dev),
        b1=np.tile(np.asarray(inputs["conv1_b"], np.float32), 4).reshape(128, 1),
        b2=np.tile(np.asarray(inputs["conv2_b"], np.float32), 2).reshape(128, 1),
        b3=np.asarray(inputs["conv3_b"], np.float32).reshape(128, 1),
        b4=np.asarray(inputs["conv4_b"], np.float32).reshape(2, 128).T.copy(),
        fb1=np.asarray(inputs["fc1_b"], np.float32).reshape(8, 128).T.copy(),
        fb2=np.asarray(inputs["fc2_b"], np.float32).reshape(29, 1),
    )
    in_maps = []
    for b in range(8):
        xb = x[b].transpose(1, 0, 2, 3).reshape(70, 6 * HW)
        m = dict(common)
        m["xin"] = bf(xb)
        in_maps.append(m)
    return in_maps


def kernel(**inputs):
    if "nc" not in _CACHE:
        _CACHE["nc"] = _build()
    nc = _CACHE["nc"]
    in_maps = _prep(inputs)
    res = run_bass_kernel_spmd(nc, in_maps, core_ids=list(range(8)))
    out = np.stack([res.results[b]["y"] for b in range(8)], axis=0)
    return out.astype(np.float32)


if __name__ == "__main__":
    pass

